# revision 16
# baseline (speedup 1.0000x reference)
"""Causal self-attention block (qkv proj + 16-head causal attention + out_proj
+ c_proj) on 8 trn2 NeuronCores, data-parallel over the batch (B=8: one batch
element per core).

Layout strategy (per core, batch element b):
  - All matmuls in bf16 (1 cycle/row on the PE at ANY moving size, unlike
    fp32r's 4x penalty below 256 rows); fp32 PSUM accumulation. End-to-end
    numpy sim of this quantization gives rel_err ~3.6e-3 vs the 2e-2 gate.
  - Activations feature-major [feature, token] so every linear layer is a
    plain  out = W_T.T @ act  matmul chain with the host-pre-transposed
    weight stationary. No on-device transposes.
  - Attention runs in NT=4 query chunks of W=256 so the tail (last chunk's
    normalize -> out_proj -> c_proj) is short; out_proj/c_proj of chunk c
    overlap attention of chunks > c.
  - Attention computes transposed scores sT[tk, tq] = k_h.T q_h per head
    pair (row-tiled K=64 matmuls), exp on ACT (no max-subtraction; scores
    are bounded), causal mask as a bf16 multiply on DVE, and AV consumes
    sT with token-major V stationary (fused ones-column yields the softmax
    denominator for free).
  - Softmax normalization: per-chunk denominators are DMA-gathered into a
    [16, W] tile, batch-reciprocated on DVE, then broadcast across the 128
    y-partitions by a K=16 indicator matmul into PSUM (no DRAM bounce, no
    per-head broadcast DMAs) and applied by DVE multiplies.
  - PSUM->SBUF drains: qk / out_proj bias-adds on ACT (Identity with
    per-partition bias), v / c_proj bias-adds on DVE (GPSIMD cannot read
    PSUM on TRN2).
"""

import sys

if "/opt/trn_rl_repo" not in sys.path:
    sys.path.insert(0, "/opt/trn_rl_repo")

import ml_dtypes
import numpy as np

import concourse.bass as bass  # noqa: F401
import concourse.tile as tile
from concourse import bacc, mybir
from concourse.bass_utils import run_bass_kernel_spmd

B, T, E, H = 8, 1024, 1024, 16
DH = E // H          # 64
JQK = 2 * E          # q+k fused feature dim (2048)
NT = 4               # attention query chunks
W = T // NT          # 256
ET = E // 128        # 8
TT = T // 128        # 8
KJ0 = JQK // 128 // 2  # 8: first k feature-tile index in qkt
F32 = mybir.dt.float32
BF16 = mybir.dt.bfloat16
Act = mybir.ActivationFunctionType

TRACE = False        # test harness flips this for profiled runs
PHASE_LIMIT = 4      # debug: 1=qk proj, 2=+v, 3=+attention, 4=full
LAG = 3              # exp->AV lag (its) so the mask multiply is off-chain
RATIO_S2 = 2         # dense matmuls interleaved per attention yield, S2
RATIO_S3 = 6         # .. S3 (dense is N=256 there, attention needs ACT time)
_CACHE = {}


def _emit(nc, tc, aps):
    (xT, wqkT, wvT, bqk, bvrow, woutT, bout, wcT, bc, mask01, onesbf, ind,
     outT) = aps

    consts = tc.alloc_tile_pool(name="consts", bufs=1)
    mask01b = consts.tile([128, 128], BF16, tag="mask01b")
    bqkb = consts.tile([128, JQK // 128], F32, tag="bqkb")
    bvb = consts.tile([128, E], BF16, tag="bvb")
    boutb = consts.tile([128, E // 128], F32, tag="boutb")
    bcb = consts.tile([128, E // 128], F32, tag="bcb")
    onesc = consts.tile([128, H], BF16, tag="onesc")
    indb = consts.tile([16, H // 2, 128], BF16, tag="indb")
    nc.sync.dma_start(out=mask01b, in_=mask01)
    nc.sync.dma_start(out=bqkb, in_=bqk)
    # v-bias broadcast [1, E] -> [128, E] via zero-stride partition read
    bsrc = bass.AP(tensor=bvrow.tensor, offset=bvrow.offset,
                   ap=[[0, 128]] + list(bvrow.ap)[1:])
    nc.sync.dma_start(out=bvb, in_=bsrc)
    nc.sync.dma_start(out=boutb, in_=bout)
    nc.sync.dma_start(out=bcb, in_=bc)
    nc.sync.dma_start(out=onesc, in_=onesbf)
    nc.sync.dma_start(out=indb, in_=ind)

    # attention PSUM: scp padded so each head-pair's accumulation lives in
    # its own 2KB bank; av0/av1 likewise separate banks (4+1+1 = 6 banks).
    # The dense-GEMM psum pools (2 banks each) swap at the S2/S3 boundary.
    psum = tc.alloc_tile_pool(name="psum", bufs=1, space="PSUM")
    p_den = tc.alloc_tile_pool(name="p_den", bufs=1)
    p_nrm = tc.alloc_tile_pool(name="p_nrm", bufs=1)
    p_y = tc.alloc_tile_pool(name="p_y", bufs=1)
    p_qk = tc.alloc_tile_pool(name="p_qk", bufs=1)
    p_v = tc.alloc_tile_pool(name="p_v", bufs=1)
    p_z = tc.alloc_tile_pool(name="p_z", bufs=1)
    p_w34 = tc.alloc_tile_pool(name="p_w34", bufs=32)
    p_x = tc.alloc_tile_pool(name="p_x", bufs=1)
    p_wqk = tc.alloc_tile_pool(name="p_wqk", bufs=16)
    psum_mm = tc.alloc_tile_pool(name="psum_mm", bufs=1, space="PSUM")

    denc = p_den.tile([16, NT, W], F32, tag="denc")
    recc = p_den.tile([16, NT, W], BF16, tag="recc")
    yt = p_y.tile([128, ET, T], BF16)
    qkt = p_qk.tile([128, JQK // 128, T], BF16)
    vt = p_v.tile([128, TT, H, DH + 1], BF16)
    zt = p_z.tile([128, ET, T], BF16)
    xt = p_x.tile([128, ET, T], BF16)

    # ---- dense generators: qkv projection ---------------------------------
    def qk_gen(jg):
        """qkT[j, t] = Wqk x^T + bqk for the 512-wide feature group jg.
        Drains on ACT for the S1 groups (ACT idle there) and DVE for the S2
        groups (ACT busy pacing attention exps)."""
        wtiles = []
        for et in range(ET):
            if jg == 0:                    # interleave x loads with group 0;
                nc.sync.dma_start(         # first token-half only, so the
                    out=xt[:, et, 0:512],  # first matmul chain isn't gated
                    in_=xT[et * 128:(et + 1) * 128, 0:512])
            wt = p_wqk.tile([128, 512], BF16, tag="wqk", name="wt")
            nc.sync.dma_start(out=wt, in_=wqkT[et * 128:(et + 1) * 128,
                                              jg * 512:(jg + 1) * 512])
            wtiles.append(wt)
        if jg == 0:
            for et in range(ET):
                nc.sync.dma_start(out=xt[:, et, 512:1024],
                                  in_=xT[et * 128:(et + 1) * 128, 512:1024])
        for js in range(4):
            jt = jg * 4 + js
            for th in range(2):
                ps = psum_mm.tile([128, 512], F32, tag="mm", bufs=2, name="pmm")
                for et in range(ET):
                    nc.tensor.matmul(
                        ps,
                        wtiles[et][:, js * 128:(js + 1) * 128],
                        xt[:, et, th * 512:(th + 1) * 512],
                        start=(et == 0), stop=(et == ET - 1))
                    yield
                if jg in (0, 2):
                    nc.scalar.activation(
                        out=qkt[:, jt, th * 512:(th + 1) * 512], in_=ps,
                        func=Act.Identity, bias=bqkb[:, jt:jt + 1], scale=1.0)
                else:
                    nc.vector.tensor_scalar_add(
                        out=qkt[:, jt, th * 512:(th + 1) * 512], in0=ps,
                        scalar1=bqkb[:, jt:jt + 1])

    def vb_gen(jh):
        """v[t, h, d] token-major for heads 8*jh..8*jh+7 (+bias on Pool),
        with a bf16 ones column at d=64 for the fused denominator."""
        if jh == 0:
            for tt in range(TT):
                nc.sync.dma_start(out=vt[:, tt, :, DH], in_=onesc)
        wvtiles = []
        for et in range(ET):
            wt = p_wqk.tile([128, 512], BF16, tag="wqk", name="wt")
            nc.sync.dma_start(out=wt, in_=wvT[et * 128:(et + 1) * 128,
                                             jh * 512:(jh + 1) * 512])
            wvtiles.append(wt)
        bvv = bvb.rearrange("p (h d) -> p h d", d=DH)
        for tt in range(TT):
            ps = psum_mm.tile([128, 512], F32, tag="mm", bufs=2, name="pmm")
            for et in range(ET):
                nc.tensor.matmul(
                    ps,
                    xt[:, et, tt * 128:(tt + 1) * 128],
                    wvtiles[et],
                    start=(et == 0), stop=(et == ET - 1))
                yield
            nc.vector.tensor_add(
                out=vt[:, tt, jh * 8:(jh + 1) * 8, 0:DH],
                in0=ps.rearrange("p (h d) -> p h d", d=DH),
                in1=bvv[:, jh * 8:(jh + 1) * 8, :])

    def w34_loader():
        """Prefetch out_proj + c_proj weights during S2 (long before use)."""
        if PHASE_LIMIT < 4:
            return
        for dst, src in ((wout_tiles, woutT), (wc_tiles, wcT)):
            for og in range(2):
                for et in range(ET):
                    wt = p_w34.tile([128, 512], BF16, tag="w34", name="wt3")
                    nc.sync.dma_start(
                        out=wt, in_=src[et * 128:(et + 1) * 128,
                                        og * 512:(og + 1) * 512])
                    dst.append(wt)
                    yield

    # ---- attention generator (yields once per tk-iteration) ---------------
    def att_gen(c, a, p_esc):
        cs = c * W
        last_it = 2 * c + 1
        avps = [psum.tile([128, W], F32, tag=f"av{p}", bufs=1,
                          name=f"avp{p}") for p in range(2)]
        pend = []

        def emit_av(it, sub, clen, esc):
            for p in range(2):
                nc.tensor.matmul(
                    avps[p][0:DH + 1, sub:sub + clen],
                    vt[:, it, 2 * a + p, :],
                    esc[:, p, :clen],
                    start=(it == 0), stop=(it == last_it),
                    skip_group_check=True)

        for it in range(last_it + 1):
            n0 = it * 128
            lo = max(n0, cs)
            sub = lo - cs
            clen = W - sub
            scp = psum.tile([128, 2, W], F32, tag="scp", bufs=2, name="scp",
                            padded_shape=[128, 2, 512])
            for p in range(2):             # paired heads: row-tiled matmuls
                pb = p * 64
                nc.tensor.matmul(
                    scp[:, p, :clen],
                    qkt[pb:pb + 64, KJ0 + a, n0:n0 + 128],
                    qkt[pb:pb + 64, a, lo:lo + clen],
                    start=True, stop=True)
            esc = p_esc.tile([128, 2, W], BF16, tag="esc", name="esc")
            nc.scalar.activation(out=esc[:, :, :clen], in_=scp[:, :, :clen],
                                 func=Act.Exp, scale=1.0 / 8.0)
            if n0 >= cs:                   # diagonal block: causal mask
                nc.vector.tensor_mul(
                    esc[:, :, 0:128], esc[:, :, 0:128],
                    mask01b[:, None, :].broadcast_to([128, 2, 128]))
            pend.append((it, sub, clen, esc))
            if len(pend) > LAG:
                emit_av(*pend.pop(0))
            yield
        for args in pend:
            emit_av(*args)
        # stage the denominator rows at partition 64 (engines address
        # partition bases in multiples of 32) and DMA-scatter (partition-
        # agnostic) into this chunk's [16, W] denominator tile.
        stg = p_nrm.tile([128, 2, W], F32, tag="stg", bufs=2, name="stg")
        for p in range(2):                 # drain unnormalized y + denom row
            nc.vector.tensor_copy(out=yt[p * 64:p * 64 + 64, a, cs:cs + W],
                                  in_=avps[p][0:DH, :])
            nc.vector.tensor_copy(out=stg[64:65, p, :],
                                  in_=avps[p][DH:DH + 1, :])
        nc.sync.dma_start(out=denc[2 * a:2 * a + 2, c, :],
                          in_=stg[64:65, :, :])

    def norm_gen(c):
        """1/denom for chunk c (batched DVE reciprocal), broadcast across
        partitions by K=16 indicator matmuls, applied in place on DVE."""
        cs = c * W
        with nc.allow_low_precision(reason="fp32 reciprocal feeding a bf16 "
                                    "multiply; well inside tolerance"):
            nc.vector.reciprocal(out=recc[:, c, :], in_=denc[:, c, :])
        yield
        for a in range(H // 2):
            rb = psum_mo.tile([128, W], F32, tag="mo", bufs=2, name="rb")
            nc.tensor.matmul(rb, indb[:, a, :], recc[:, c, :],
                             start=True, stop=True)
            yield
            nc.vector.tensor_mul(yt[:, a, cs:cs + W], yt[:, a, cs:cs + W],
                                 rb)
            yield

    def oproj_gen(c):
        cs = c * W
        for ot in range(ET):
            og, os_ = divmod(ot, 4)
            ps = psum_mo.tile([128, W], F32, tag="mo", bufs=2, name="po")
            for et in range(ET):
                nc.tensor.matmul(
                    ps,
                    wout_tiles[og * ET + et][:, os_ * 128:(os_ + 1) * 128],
                    yt[:, et, cs:cs + W],
                    start=(et == 0), stop=(et == ET - 1))
                yield
            nc.scalar.activation(out=zt[:, ot, cs:cs + W], in_=ps,
                                 func=Act.Identity, bias=boutb[:, ot:ot + 1],
                                 scale=1.0)

    def cproj_gen(c):
        cs = c * W
        for ot in range(ET):
            og, os_ = divmod(ot, 4)
            ps = psum_mo.tile([128, W], F32, tag="mo", bufs=2, name="pc")
            for et in range(ET):
                nc.tensor.matmul(
                    ps,
                    wc_tiles[og * ET + et][:, os_ * 128:(os_ + 1) * 128],
                    zt[:, et, cs:cs + W],
                    start=(et == 0), stop=(et == ET - 1))
                yield
            ob = p_out.tile([128, W], F32, tag="ob", bufs=3, name="ob")
            nc.vector.tensor_scalar_add(out=ob, in0=ps,
                                        scalar1=bcb[:, ot:ot + 1])
            nc.sync.dma_start(out=outT[ot * 128:(ot + 1) * 128, cs:cs + W],
                              in_=ob)

    # ---- drivers ----------------------------------------------------------
    def run_dense(dense, n=None):
        steps = 0
        while dense and (n is None or steps < n):
            try:
                next(dense[0])
                steps += 1
            except StopIteration:
                dense.pop(0)
        return steps

    def drive(att_units, dense, ratio):
        att_units = list(att_units)
        while att_units:
            try:
                next(att_units[0])
            except StopIteration:
                att_units.pop(0)
                continue
            run_dense(dense, ratio)
        run_dense(dense)

    wout_tiles = []
    wc_tiles = []

    # S1: dense-only warmup — deps for attention pairs 0-3
    dense1 = [qk_gen(0), qk_gen(2)] + ([vb_gen(0)] if PHASE_LIMIT >= 2 else [])
    run_dense(dense1)

    # S2: attention pairs 0-3 (all chunks) over the remaining qkv work,
    # with out/c_proj weight prefetch at the back of the DMA queue
    p_esc1 = tc.alloc_tile_pool(name="p_esc1", bufs=6)
    dense2 = [qk_gen(1), qk_gen(3)] + ([vb_gen(1)] if PHASE_LIMIT >= 2 else [])
    dense2.append(w34_loader())
    att2 = [att_gen(c, a, p_esc1)
            for a in range(4) for c in range(NT)] if PHASE_LIMIT >= 3 else []
    drive(att2, dense2, RATIO_S2)
    p_esc1.release()
    psum_mm.release()
    p_wqk.release()
    p_x.release()

    # S3/S4: attention pairs 4-7 chunk by chunk; after each chunk completes,
    # its normalize + out_proj + c_proj join the dense stream
    psum_mo = tc.alloc_tile_pool(name="psum_mo", bufs=1, space="PSUM")
    p_esc2 = tc.alloc_tile_pool(name="p_esc2", bufs=6)
    p_out = tc.alloc_tile_pool(name="p_out", bufs=3)
    dense3 = []
    if PHASE_LIMIT >= 3:
        for c in range(NT):
            drive([att_gen(c, a, p_esc2) for a in range(4, 8)],
                  dense3, RATIO_S3)
            if PHASE_LIMIT >= 4:
                dense3.append(norm_gen(c))
                dense3.append(oproj_gen(c))
                dense3.append(cproj_gen(c))
    run_dense(dense3)

    p_out.release()
    p_esc2.release()
    psum_mo.release()
    p_w34.release()
    p_z.release()
    p_v.release()
    p_qk.release()
    p_y.release()
    p_nrm.release()
    p_den.release()
    psum.release()
    consts.release()


def _build():
    if "nc" in _CACHE:
        return _CACHE["nc"]
    nc = bacc.Bacc("TRN2", target_bir_lowering=False, debug=False,
                   enable_asserts=True, num_devices=8)
    d = nc.dram_tensor
    aps = [
        d("xT", [E, T], BF16, kind="ExternalInput").ap(),
        d("wqkT", [E, JQK], BF16, kind="ExternalInput").ap(),
        d("wvT", [E, E], BF16, kind="ExternalInput").ap(),
        d("bqk", [128, JQK // 128], F32, kind="ExternalInput").ap(),
        d("bvrow", [1, E], BF16, kind="ExternalInput").ap(),
        d("woutT", [E, E], BF16, kind="ExternalInput").ap(),
        d("bout", [128, E // 128], F32, kind="ExternalInput").ap(),
        d("wcT", [E, E], BF16, kind="ExternalInput").ap(),
        d("bc", [128, E // 128], F32, kind="ExternalInput").ap(),
        d("mask01", [128, 128], BF16, kind="ExternalInput").ap(),
        d("onesbf", [128, H], BF16, kind="ExternalInput").ap(),
        d("ind", [16, (H // 2) * 128], BF16, kind="ExternalInput").ap(),
        d("outT", [E, T], F32, kind="ExternalOutput").ap(),
    ]
    with tile.TileContext(nc) as tc:
        _emit(nc, tc, aps)
    nc.compile()
    _CACHE["nc"] = nc
    return nc


def _host_inputs(x, in_proj_w, in_proj_b, out_proj_w, out_proj_b,
                 c_proj_w, c_proj_b):
    f = np.float32
    bf = ml_dtypes.bfloat16
    x = np.asarray(x, f)
    in_proj_w = np.asarray(in_proj_w, f)
    in_proj_b = np.asarray(in_proj_b, f)
    ind = np.zeros((16, H // 2, 128), f)
    for a in range(H // 2):
        ind[2 * a, a, 0:64] = 1.0
        ind[2 * a + 1, a, 64:128] = 1.0
    shared = {
        "wqkT": np.ascontiguousarray(in_proj_w[:JQK].T).astype(bf),
        "wvT": np.ascontiguousarray(in_proj_w[JQK:].T).astype(bf),
        "bqk": np.ascontiguousarray(
            in_proj_b[:JQK].reshape(JQK // 128, 128).T),
        "bvrow": in_proj_b[JQK:].reshape(1, E).astype(bf),
        "woutT": np.ascontiguousarray(np.asarray(out_proj_w, f).T).astype(bf),
        "bout": np.ascontiguousarray(
            np.asarray(out_proj_b, f).reshape(E // 128, 128).T),
        "wcT": np.ascontiguousarray(np.asarray(c_proj_w, f).T).astype(bf),
        "bc": np.ascontiguousarray(
            np.asarray(c_proj_b, f).reshape(E // 128, 128).T),
        "mask01": np.where(np.arange(128)[None, :] >= np.arange(128)[:, None],
                           f(1.0), f(0.0)).astype(bf),
        "onesbf": np.ones((128, H), bf),
        "ind": np.ascontiguousarray(ind.reshape(16, (H // 2) * 128)).astype(
            bf),
    }
    return [{**shared, "xT": np.ascontiguousarray(x[b].T).astype(bf)}
            for b in range(B)]


def kernel(x, in_proj_w, in_proj_b, out_proj_w, out_proj_b, c_proj_w,
           c_proj_b):
    nc = _build()
    in_maps = _host_inputs(x, in_proj_w, in_proj_b, out_proj_w, out_proj_b,
                           c_proj_w, c_proj_b)
    res = run_bass_kernel_spmd(nc, in_maps, core_ids=list(range(B)),
                               trace=TRACE)
    _CACHE["last_result"] = res
    out = np.stack([res.results[b]["outT"].T for b in range(B)])
    return np.ascontiguousarray(out, dtype=np.float32)


# revision 26
# speedup vs baseline: 1.0424x; 1.0424x over previous
"""Causal self-attention block (qkv proj + 16-head causal attention + out_proj
+ c_proj) on 8 trn2 NeuronCores, data-parallel over the batch (B=8: one batch
element per core).

Layout strategy (per core, batch element b):
  - All matmuls in bf16 (1 cycle/row on the PE at ANY moving size, unlike
    fp32r's 4x penalty below 256 rows); fp32 PSUM accumulation. End-to-end
    numpy sim of this quantization gives rel_err ~3.6e-3 vs the 2e-2 gate.
  - Activations feature-major [feature, token] so every linear layer is a
    plain  out = W_T.T @ act  matmul chain with the host-pre-transposed
    weight stationary. No on-device transposes.
  - Attention runs in NT=4 query chunks of W=256 so the tail (last chunk's
    normalize -> out_proj -> c_proj) is short; out_proj/c_proj of chunk c
    overlap attention of chunks > c.
  - Attention computes transposed scores sT[tk, tq] = k_h.T q_h per head
    pair (row-tiled K=64 matmuls), exp on ACT (no max-subtraction; scores
    are bounded), causal mask as a bf16 multiply on DVE, and AV consumes
    sT with token-major V stationary (fused ones-column yields the softmax
    denominator for free).
  - Softmax normalization: per-chunk denominators are DMA-gathered into a
    [16, W] tile, batch-reciprocated on DVE, then broadcast across the 128
    y-partitions by a K=16 indicator matmul into PSUM (no DRAM bounce, no
    per-head broadcast DMAs) and applied by DVE multiplies.
  - PSUM->SBUF drains: qk / out_proj bias-adds on ACT (Identity with
    per-partition bias), v / c_proj bias-adds on DVE (GPSIMD cannot read
    PSUM on TRN2).
"""

import sys

if "/opt/trn_rl_repo" not in sys.path:
    sys.path.insert(0, "/opt/trn_rl_repo")

import ml_dtypes
import numpy as np

import concourse.bass as bass  # noqa: F401
import concourse.tile as tile
from concourse import bacc, mybir
from concourse.bass_utils import run_bass_kernel_spmd

B, T, E, H = 8, 1024, 1024, 16
DH = E // H          # 64
JQK = 2 * E          # q+k fused feature dim (2048)
NT = 4               # attention query chunks
W = T // NT          # 256
ET = E // 128        # 8
TT = T // 128        # 8
KJ0 = JQK // 128 // 2  # 8: first k feature-tile index in qkt
F32 = mybir.dt.float32
BF16 = mybir.dt.bfloat16
Act = mybir.ActivationFunctionType

TRACE = False        # test harness flips this for profiled runs
PHASE_LIMIT = 4      # debug: 1=qk proj, 2=+v, 3=+attention, 4=full
LAG = 3              # exp->AV lag (its) so the mask multiply is off-chain
RATIO_S2 = 2         # dense matmuls interleaved per attention yield, S2
RATIO_S3 = 5         # .. S3 (dense is N=256 there, attention needs ACT time)
_CACHE = {}


def _emit(nc, tc, aps):
    (xT, wqkT, wvT, bqk, bvrow, woutT, bout, wcT, bc, mask01, onesbf, ind,
     outT) = aps

    consts = tc.alloc_tile_pool(name="consts", bufs=1)
    mask01b = consts.tile([128, 128], BF16, tag="mask01b")
    bqkb = consts.tile([128, JQK // 128], F32, tag="bqkb")
    bvb = consts.tile([128, E], BF16, tag="bvb")
    boutb = consts.tile([128, E // 128], F32, tag="boutb")
    bcb = consts.tile([128, E // 128], F32, tag="bcb")
    onesc = consts.tile([128, H], BF16, tag="onesc")
    indb = consts.tile([16, H // 2, 128], BF16, tag="indb")
    # small consts up front (~0.2 us); the larger bvb/indb are deferred to
    # vb_gen(0) so they don't delay the first matmul chain's x/w loads
    nc.sync.dma_start(out=mask01b, in_=mask01)
    nc.sync.dma_start(out=bqkb, in_=bqk)
    nc.sync.dma_start(out=boutb, in_=bout)
    nc.sync.dma_start(out=bcb, in_=bc)
    nc.sync.dma_start(out=onesc, in_=onesbf)

    # attention PSUM: scp padded so each head-pair's accumulation lives in
    # its own 2KB bank; av0/av1 likewise separate banks (4+1+1 = 6 banks).
    # The dense-GEMM psum pools (2 banks each) swap at the S2/S3 boundary.
    psum = tc.alloc_tile_pool(name="psum", bufs=1, space="PSUM")
    p_den = tc.alloc_tile_pool(name="p_den", bufs=1)
    p_nrm = tc.alloc_tile_pool(name="p_nrm", bufs=1)
    p_y = tc.alloc_tile_pool(name="p_y", bufs=1)
    p_qk = tc.alloc_tile_pool(name="p_qk", bufs=1)
    p_v = tc.alloc_tile_pool(name="p_v", bufs=1)
    p_z = tc.alloc_tile_pool(name="p_z", bufs=1)
    p_w34 = tc.alloc_tile_pool(name="p_w34", bufs=32)
    p_x = tc.alloc_tile_pool(name="p_x", bufs=1)
    p_wqk = tc.alloc_tile_pool(name="p_wqk", bufs=16)
    psum_mm = tc.alloc_tile_pool(name="psum_mm", bufs=1, space="PSUM")

    denc = p_den.tile([16, NT, W], F32, tag="denc")
    recc = p_den.tile([16, NT, W], BF16, tag="recc")
    yt = p_y.tile([128, ET, T], BF16)
    qkt = p_qk.tile([128, JQK // 128, T], BF16)
    vt = p_v.tile([128, TT, H, DH + 1], BF16)
    zt = p_z.tile([128, ET, T], BF16)
    xt = p_x.tile([128, ET, T], BF16)

    # ---- dense generators: qkv projection ---------------------------------
    xTr = xT.rearrange("(e p) t -> p e t", p=128)

    def qk_gen(jg):
        """qkT[j, t] = Wqk x^T + bqk for the 512-wide feature group jg.
        All 8 e-tiles of a weight group load as ONE 1MB DMA (per-transfer
        overhead makes 16 small DMAs ~2x slower than the bytes warrant).
        Drains on ACT for the S1 groups (ACT idle there) and DVE for the S2
        groups (ACT busy pacing attention exps)."""
        if jg == 0:                        # x token-halves: first half now so
            nc.sync.dma_start(             # the first chain isn't gated on
                out=xt[:, :, 0:512],       # the full 2MB
                in_=xTr[:, :, 0:512])
        wg = p_wqk.tile([128, ET, 512], BF16, tag="wqk", bufs=2, name="wg")
        nc.sync.dma_start(
            out=wg, in_=wqkT[:, jg * 512:(jg + 1) * 512].rearrange(
                "(e p) j -> p e j", p=128))
        if jg == 0:
            nc.sync.dma_start(out=xt[:, :, 512:1024], in_=xTr[:, :, 512:1024])
        for th in range(2):
            for js in range(4):
                jt = jg * 4 + js
                ps = psum_mm.tile([128, 512], F32, tag="mm", bufs=2, name="pmm")
                for et in range(ET):
                    nc.tensor.matmul(
                        ps,
                        wg[:, et, js * 128:(js + 1) * 128],
                        xt[:, et, th * 512:(th + 1) * 512],
                        start=(et == 0), stop=(et == ET - 1))
                    yield
                if jg in (0, 2):
                    nc.scalar.activation(
                        out=qkt[:, jt, th * 512:(th + 1) * 512], in_=ps,
                        func=Act.Identity, bias=bqkb[:, jt:jt + 1], scale=1.0)
                else:
                    nc.vector.tensor_scalar_add(
                        out=qkt[:, jt, th * 512:(th + 1) * 512], in0=ps,
                        scalar1=bqkb[:, jt:jt + 1])

    def vb_gen(jh):
        """v[t, h, d] token-major for heads 8*jh..8*jh+7 (+bias on DVE),
        with a bf16 ones column at d=64 for the fused denominator."""
        if jh == 0:
            # deferred large consts: v-bias broadcast [1, E] -> [128, E] via
            # zero-stride partition read, and the indicator matrix
            bsrc = bass.AP(tensor=bvrow.tensor, offset=bvrow.offset,
                           ap=[[0, 128]] + list(bvrow.ap)[1:])
            nc.sync.dma_start(out=bvb, in_=bsrc)
            nc.sync.dma_start(out=indb, in_=ind)
            for tt in range(TT):
                nc.sync.dma_start(out=vt[:, tt, :, DH], in_=onesc)
        wvg = p_wqk.tile([128, ET, 512], BF16, tag="wqk", bufs=2, name="wvg")
        nc.sync.dma_start(
            out=wvg, in_=wvT[:, jh * 512:(jh + 1) * 512].rearrange(
                "(e p) j -> p e j", p=128))
        bvv = bvb.rearrange("p (h d) -> p h d", d=DH)
        for tt in range(TT):
            ps = psum_mm.tile([128, 512], F32, tag="mm", bufs=2, name="pmm")
            for et in range(ET):
                nc.tensor.matmul(
                    ps,
                    xt[:, et, tt * 128:(tt + 1) * 128],
                    wvg[:, et, :],
                    start=(et == 0), stop=(et == ET - 1))
                yield
            nc.vector.tensor_add(
                out=vt[:, tt, jh * 8:(jh + 1) * 8, 0:DH],
                in0=ps.rearrange("p (h d) -> p h d", d=DH),
                in1=bvv[:, jh * 8:(jh + 1) * 8, :])

    def w34_loader():
        """Prefetch out_proj + c_proj weights during S2 (long before use),
        one 2MB DMA per matrix."""
        if PHASE_LIMIT < 4:
            return
        for dst, src in ((wout_tiles, woutT), (wc_tiles, wcT)):
            for og in range(2):
                wt = p_w34.tile([128, ET, 512], BF16, tag="w34", bufs=4,
                                name="wt3")
                nc.sync.dma_start(
                    out=wt, in_=src[:, og * 512:(og + 1) * 512].rearrange(
                        "(e p) j -> p e j", p=128))
                dst.append(wt)
                yield

    # ---- attention generator (yields once per tk-iteration) ---------------
    def att_gen(c, a, p_esc):
        cs = c * W
        last_it = 2 * c + 1
        avps = [psum.tile([128, W], F32, tag=f"av{p}", bufs=1,
                          name=f"avp{p}") for p in range(2)]
        pend = []

        def emit_av(it, sub, clen, esc):
            for p in range(2):
                nc.tensor.matmul(
                    avps[p][0:DH + 1, sub:sub + clen],
                    vt[:, it, 2 * a + p, :],
                    esc[:, p, :clen],
                    start=(it == 0), stop=(it == last_it),
                    skip_group_check=True)

        for it in range(last_it + 1):
            n0 = it * 128
            lo = max(n0, cs)
            sub = lo - cs
            clen = W - sub
            scp = psum.tile([128, 2, W], F32, tag="scp", bufs=2, name="scp",
                            padded_shape=[128, 2, 512])
            for p in range(2):             # paired heads: row-tiled matmuls
                pb = p * 64
                nc.tensor.matmul(
                    scp[:, p, :clen],
                    qkt[pb:pb + 64, KJ0 + a, n0:n0 + 128],
                    qkt[pb:pb + 64, a, lo:lo + clen],
                    start=True, stop=True)
            esc = p_esc.tile([128, 2, W], BF16, tag="esc", name="esc")
            nc.scalar.activation(out=esc[:, :, :clen], in_=scp[:, :, :clen],
                                 func=Act.Exp, scale=1.0 / 8.0)
            if n0 >= cs:                   # diagonal block: causal mask
                nc.vector.tensor_mul(
                    esc[:, :, 0:128], esc[:, :, 0:128],
                    mask01b[:, None, :].broadcast_to([128, 2, 128]))
            pend.append((it, sub, clen, esc))
            if len(pend) > LAG:
                emit_av(*pend.pop(0))
            yield
        for args in pend:
            emit_av(*args)
        # stage the denominator rows at partition 64 (engines address
        # partition bases in multiples of 32) and DMA-scatter (partition-
        # agnostic) into this chunk's [16, W] denominator tile.
        stg = p_nrm.tile([128, 2, W], F32, tag="stg", bufs=2, name="stg")
        for p in range(2):                 # drain unnormalized y + denom row
            nc.vector.tensor_copy(out=yt[p * 64:p * 64 + 64, a, cs:cs + W],
                                  in_=avps[p][0:DH, :])
            nc.vector.tensor_copy(out=stg[64:65, p, :],
                                  in_=avps[p][DH:DH + 1, :])
        nc.sync.dma_start(out=denc[2 * a:2 * a + 2, c, :],
                          in_=stg[64:65, :, :])

    def emit_rec(c):
        """1/denom for chunk c (batched DVE reciprocal). Emitted directly
        (not via the dense FIFO) the moment chunk c's attention completes,
        so the reciprocal latency hides under leftover dense matmuls."""
        with nc.allow_low_precision(reason="fp32 reciprocal feeding a bf16 "
                                    "multiply; well inside tolerance"):
            nc.vector.reciprocal(out=recc[:, c, :], in_=denc[:, c, :])

    def norm_gen(c):
        """Broadcast 1/denom across partitions by K=16 indicator matmuls,
        apply in place on DVE."""
        cs = c * W
        for a in range(H // 2):
            rb = psum_mo.tile([128, W], F32, tag="mo", bufs=2, name="rb")
            nc.tensor.matmul(rb, indb[:, a, :], recc[:, c, :],
                             start=True, stop=True)
            yield
            nc.vector.tensor_mul(yt[:, a, cs:cs + W], yt[:, a, cs:cs + W],
                                 rb)
            yield

    def oproj_gen(c):
        cs = c * W
        for ot in range(ET):
            og, os_ = divmod(ot, 4)
            ps = psum_mo.tile([128, W], F32, tag="mo", bufs=2, name="po")
            for et in range(ET):
                nc.tensor.matmul(
                    ps,
                    wout_tiles[og][:, et, os_ * 128:(os_ + 1) * 128],
                    yt[:, et, cs:cs + W],
                    start=(et == 0), stop=(et == ET - 1))
                yield
            nc.scalar.activation(out=zt[:, ot, cs:cs + W], in_=ps,
                                 func=Act.Identity, bias=boutb[:, ot:ot + 1],
                                 scale=1.0)

    def cproj_gen(c):
        cs = c * W
        for ot in range(ET):
            og, os_ = divmod(ot, 4)
            ps = psum_mo.tile([128, W], F32, tag="mo", bufs=2, name="pc")
            for et in range(ET):
                nc.tensor.matmul(
                    ps,
                    wc_tiles[og][:, et, os_ * 128:(os_ + 1) * 128],
                    zt[:, et, cs:cs + W],
                    start=(et == 0), stop=(et == ET - 1))
                yield
            ob = p_out.tile([128, W], F32, tag="ob", bufs=3, name="ob")
            nc.vector.tensor_scalar_add(out=ob, in0=ps,
                                        scalar1=bcb[:, ot:ot + 1])
            nc.sync.dma_start(out=outT[ot * 128:(ot + 1) * 128, cs:cs + W],
                              in_=ob)

    # ---- drivers ----------------------------------------------------------
    def run_dense(dense, n=None):
        steps = 0
        while dense and (n is None or steps < n):
            try:
                next(dense[0])
                steps += 1
            except StopIteration:
                dense.pop(0)
        return steps

    def drive(att_units, dense, ratio):
        att_units = list(att_units)
        while att_units:
            try:
                next(att_units[0])
            except StopIteration:
                att_units.pop(0)
                continue
            run_dense(dense, ratio)
        run_dense(dense)

    wout_tiles = []
    wc_tiles = []

    # S1: dense-only warmup — deps for attention pairs 0-3
    dense1 = [qk_gen(0), qk_gen(2)] + ([vb_gen(0)] if PHASE_LIMIT >= 2 else [])
    run_dense(dense1)

    # S2: attention pairs 0-3 (all chunks) over the remaining qkv work,
    # with out/c_proj weight prefetch at the back of the DMA queue
    p_esc1 = tc.alloc_tile_pool(name="p_esc1", bufs=6)
    dense2 = [qk_gen(1), qk_gen(3)] + ([vb_gen(1)] if PHASE_LIMIT >= 2 else [])
    dense2.append(w34_loader())
    att2 = [att_gen(c, a, p_esc1)
            for a in range(4) for c in range(NT)] if PHASE_LIMIT >= 3 else []
    drive(att2, dense2, RATIO_S2)
    p_esc1.release()
    psum_mm.release()
    p_wqk.release()
    p_x.release()

    # S3/S4: attention pairs 4-7 chunk by chunk; after each chunk completes,
    # its normalize + out_proj + c_proj join the dense stream
    psum_mo = tc.alloc_tile_pool(name="psum_mo", bufs=1, space="PSUM")
    p_esc2 = tc.alloc_tile_pool(name="p_esc2", bufs=6)
    p_out = tc.alloc_tile_pool(name="p_out", bufs=3)
    dense3 = []
    if PHASE_LIMIT >= 3:
        for c in range(NT):
            drive([att_gen(c, a, p_esc2) for a in range(4, 8)],
                  dense3, RATIO_S3)
            if PHASE_LIMIT >= 4:
                emit_rec(c)
                dense3.append(norm_gen(c))
                dense3.append(oproj_gen(c))
                dense3.append(cproj_gen(c))
    run_dense(dense3)

    p_out.release()
    p_esc2.release()
    psum_mo.release()
    p_w34.release()
    p_z.release()
    p_v.release()
    p_qk.release()
    p_y.release()
    p_nrm.release()
    p_den.release()
    psum.release()
    consts.release()


def _build():
    if "nc" in _CACHE:
        return _CACHE["nc"]
    nc = bacc.Bacc("TRN2", target_bir_lowering=False, debug=False,
                   enable_asserts=True, num_devices=8)
    d = nc.dram_tensor
    aps = [
        d("xT", [E, T], BF16, kind="ExternalInput").ap(),
        d("wqkT", [E, JQK], BF16, kind="ExternalInput").ap(),
        d("wvT", [E, E], BF16, kind="ExternalInput").ap(),
        d("bqk", [128, JQK // 128], F32, kind="ExternalInput").ap(),
        d("bvrow", [1, E], BF16, kind="ExternalInput").ap(),
        d("woutT", [E, E], BF16, kind="ExternalInput").ap(),
        d("bout", [128, E // 128], F32, kind="ExternalInput").ap(),
        d("wcT", [E, E], BF16, kind="ExternalInput").ap(),
        d("bc", [128, E // 128], F32, kind="ExternalInput").ap(),
        d("mask01", [128, 128], BF16, kind="ExternalInput").ap(),
        d("onesbf", [128, H], BF16, kind="ExternalInput").ap(),
        d("ind", [16, (H // 2) * 128], BF16, kind="ExternalInput").ap(),
        d("outT", [E, T], F32, kind="ExternalOutput").ap(),
    ]
    with tile.TileContext(nc) as tc:
        _emit(nc, tc, aps)
    nc.compile()
    _CACHE["nc"] = nc
    return nc


def _host_inputs(x, in_proj_w, in_proj_b, out_proj_w, out_proj_b,
                 c_proj_w, c_proj_b):
    f = np.float32
    bf = ml_dtypes.bfloat16
    x = np.asarray(x, f)
    in_proj_w = np.asarray(in_proj_w, f)
    in_proj_b = np.asarray(in_proj_b, f)
    ind = np.zeros((16, H // 2, 128), f)
    for a in range(H // 2):
        ind[2 * a, a, 0:64] = 1.0
        ind[2 * a + 1, a, 64:128] = 1.0
    shared = {
        "wqkT": np.ascontiguousarray(in_proj_w[:JQK].T).astype(bf),
        "wvT": np.ascontiguousarray(in_proj_w[JQK:].T).astype(bf),
        "bqk": np.ascontiguousarray(
            in_proj_b[:JQK].reshape(JQK // 128, 128).T),
        "bvrow": in_proj_b[JQK:].reshape(1, E).astype(bf),
        "woutT": np.ascontiguousarray(np.asarray(out_proj_w, f).T).astype(bf),
        "bout": np.ascontiguousarray(
            np.asarray(out_proj_b, f).reshape(E // 128, 128).T),
        "wcT": np.ascontiguousarray(np.asarray(c_proj_w, f).T).astype(bf),
        "bc": np.ascontiguousarray(
            np.asarray(c_proj_b, f).reshape(E // 128, 128).T),
        "mask01": np.where(np.arange(128)[None, :] >= np.arange(128)[:, None],
                           f(1.0), f(0.0)).astype(bf),
        "onesbf": np.ones((128, H), bf),
        "ind": np.ascontiguousarray(ind.reshape(16, (H // 2) * 128)).astype(
            bf),
    }
    return [{**shared, "xT": np.ascontiguousarray(x[b].T).astype(bf)}
            for b in range(B)]


def kernel(x, in_proj_w, in_proj_b, out_proj_w, out_proj_b, c_proj_w,
           c_proj_b):
    nc = _build()
    in_maps = _host_inputs(x, in_proj_w, in_proj_b, out_proj_w, out_proj_b,
                           c_proj_w, c_proj_b)
    res = run_bass_kernel_spmd(nc, in_maps, core_ids=list(range(B)),
                               trace=TRACE)
    _CACHE["last_result"] = res
    out = np.stack([res.results[b]["outT"].T for b in range(B)])
    return np.ascontiguousarray(out, dtype=np.float32)


# revision 30
# speedup vs baseline: 1.0512x; 1.0085x over previous
"""Causal self-attention block (qkv proj + 16-head causal attention + out_proj
+ c_proj) on 8 trn2 NeuronCores, data-parallel over the batch (B=8: one batch
element per core).

Layout strategy (per core, batch element b):
  - All matmuls in bf16 (1 cycle/row on the PE at ANY moving size, unlike
    fp32r's 4x penalty below 256 rows); fp32 PSUM accumulation. End-to-end
    numpy sim of this quantization gives rel_err ~3.6e-3 vs the 2e-2 gate.
  - Activations feature-major [feature, token] so every linear layer is a
    plain  out = W_T.T @ act  matmul chain with the host-pre-transposed
    weight stationary. No on-device transposes.
  - Attention runs in NT=4 query chunks of W=256 so the tail (last chunk's
    normalize -> out_proj -> c_proj) is short; out_proj/c_proj of chunk c
    overlap attention of chunks > c.
  - Attention computes transposed scores sT[tk, tq] = k_h.T q_h per head
    pair (row-tiled K=64 matmuls), exp on ACT (no max-subtraction; scores
    are bounded), causal mask as a bf16 multiply on DVE, and AV consumes
    sT with token-major V stationary (fused ones-column yields the softmax
    denominator for free).
  - Softmax normalization: per-chunk denominators are DMA-gathered into a
    [16, W] tile, batch-reciprocated on DVE, then broadcast across the 128
    y-partitions by a K=16 indicator matmul into PSUM (no DRAM bounce, no
    per-head broadcast DMAs) and applied by DVE multiplies.
  - PSUM->SBUF drains: qk / out_proj bias-adds on ACT (Identity with
    per-partition bias), v / c_proj bias-adds on DVE (GPSIMD cannot read
    PSUM on TRN2).
"""

import sys

if "/opt/trn_rl_repo" not in sys.path:
    sys.path.insert(0, "/opt/trn_rl_repo")

import ml_dtypes
import numpy as np

import concourse.bass as bass  # noqa: F401
import concourse.tile as tile
from concourse import bacc, mybir
from concourse.bass_utils import run_bass_kernel_spmd

B, T, E, H = 8, 1024, 1024, 16
DH = E // H          # 64
JQK = 2 * E          # q+k fused feature dim (2048)
NT = 4               # attention query chunks
W = T // NT          # 256
ET = E // 128        # 8
TT = T // 128        # 8
KJ0 = JQK // 128 // 2  # 8: first k feature-tile index in qkt
F32 = mybir.dt.float32
BF16 = mybir.dt.bfloat16
Act = mybir.ActivationFunctionType

TRACE = False        # test harness flips this for profiled runs
PHASE_LIMIT = 4      # debug: 1=qk proj, 2=+v, 3=+attention, 4=full
LAG = 3              # exp->AV lag (its) so the mask multiply is off-chain
RATIO_S2 = 2         # dense matmuls interleaved per attention yield, S2
RATIO_S3 = 5         # .. S3 (dense is N=256 there, attention needs ACT time)
_CACHE = {}


def _emit(nc, tc, aps):
    (xT, wqkT, wvT, bqk, bvrow, woutT, bout, wcT, bc, mask01, onesbf, ind,
     outT) = aps

    consts = tc.alloc_tile_pool(name="consts", bufs=1)
    mask01b = consts.tile([128, 128], BF16, tag="mask01b")
    bqkb = consts.tile([128, JQK // 128], F32, tag="bqkb")
    bvb = consts.tile([128, E], BF16, tag="bvb")
    boutb = consts.tile([128, E // 128], F32, tag="boutb")
    bcb = consts.tile([128, E // 128], F32, tag="bcb")
    onesc = consts.tile([128, H], BF16, tag="onesc")
    indb = consts.tile([8, H // 4, 128], BF16, tag="indb")
    # consts are DMA'd inside qk_gen(0)/vb_gen(0), after the time-critical
    # first x/weight loads

    # attention PSUM: scp padded so each head-pair's accumulation lives in
    # its own 2KB bank; av0/av1 likewise separate banks (4+1+1 = 6 banks).
    # The dense-GEMM psum pools (2 banks each) swap at the S2/S3 boundary.
    psum = tc.alloc_tile_pool(name="psum", bufs=1, space="PSUM")
    p_den = tc.alloc_tile_pool(name="p_den", bufs=1)
    p_nrm = tc.alloc_tile_pool(name="p_nrm", bufs=1)
    p_y = tc.alloc_tile_pool(name="p_y", bufs=1)
    p_qk = tc.alloc_tile_pool(name="p_qk", bufs=1)
    p_v = tc.alloc_tile_pool(name="p_v", bufs=1)
    p_z = tc.alloc_tile_pool(name="p_z", bufs=1)
    p_w34 = tc.alloc_tile_pool(name="p_w34", bufs=32)
    p_x = tc.alloc_tile_pool(name="p_x", bufs=1)
    p_wqk = tc.alloc_tile_pool(name="p_wqk", bufs=16)
    psum_mm = tc.alloc_tile_pool(name="psum_mm", bufs=1, space="PSUM")

    denc = [p_den.tile([8, NT, W], F32, tag=f"denc{h}", name=f"denc{h}")
            for h in range(2)]
    recc = [p_den.tile([8, NT, W], BF16, tag=f"recc{h}", name=f"recc{h}")
            for h in range(2)]
    yt = p_y.tile([128, ET, T], BF16)
    qkt = p_qk.tile([128, JQK // 128, T], BF16)
    vt = p_v.tile([128, TT, H, DH + 1], BF16)
    zt = p_z.tile([128, ET, T], BF16)
    xt = p_x.tile([128, ET, T], BF16)

    # ---- dense generators: qkv projection ---------------------------------
    xTr = xT.rearrange("(e p) t -> p e t", p=128)

    def qk_gen(jg):
        """qkT[j, t] = Wqk x^T + bqk for the 512-wide feature group jg.
        All 8 e-tiles of a weight group load as ONE 1MB DMA (per-transfer
        overhead makes 16 small DMAs ~2x slower than the bytes warrant).
        Drains on ACT for the S1 groups (ACT idle there) and DVE for the S2
        groups (ACT busy pacing attention exps)."""
        if jg == 0:                        # x on the ACT hwdge queue so it
            nc.scalar.dma_start(           # loads concurrently with the
                out=xt[:, :, 0:512],       # weight group on the SP queue
                in_=xTr[:, :, 0:512])
        wg = p_wqk.tile([128, ET, 512], BF16, tag="wqk", bufs=2, name="wg")
        nc.sync.dma_start(
            out=wg, in_=wqkT[:, jg * 512:(jg + 1) * 512].rearrange(
                "(e p) j -> p e j", p=128))
        if jg == 0:
            nc.scalar.dma_start(out=xt[:, :, 512:1024],
                                in_=xTr[:, :, 512:1024])
            nc.sync.dma_start(out=bqkb, in_=bqk)
            nc.sync.dma_start(out=mask01b, in_=mask01)
            nc.sync.dma_start(out=boutb, in_=bout)
            nc.sync.dma_start(out=bcb, in_=bc)
            nc.sync.dma_start(out=onesc, in_=onesbf)
        for th in range(2):
            for js in range(4):
                jt = jg * 4 + js
                ps = psum_mm.tile([128, 512], F32, tag="mm", bufs=2, name="pmm")
                for et in range(ET):
                    nc.tensor.matmul(
                        ps,
                        wg[:, et, js * 128:(js + 1) * 128],
                        xt[:, et, th * 512:(th + 1) * 512],
                        start=(et == 0), stop=(et == ET - 1))
                    yield
                if jg in (0, 2):
                    nc.scalar.activation(
                        out=qkt[:, jt, th * 512:(th + 1) * 512], in_=ps,
                        func=Act.Identity, bias=bqkb[:, jt:jt + 1], scale=1.0)
                else:
                    nc.vector.tensor_scalar_add(
                        out=qkt[:, jt, th * 512:(th + 1) * 512], in0=ps,
                        scalar1=bqkb[:, jt:jt + 1])

    def vb_gen(jh):
        """v[t, h, d] token-major for heads 8*jh..8*jh+7 (+bias on DVE),
        with a bf16 ones column at d=64 for the fused denominator."""
        if jh == 0:
            # deferred large consts: v-bias broadcast [1, E] -> [128, E] via
            # zero-stride partition read, and the indicator matrix
            bsrc = bass.AP(tensor=bvrow.tensor, offset=bvrow.offset,
                           ap=[[0, 128]] + list(bvrow.ap)[1:])
            nc.sync.dma_start(out=bvb, in_=bsrc)
            nc.sync.dma_start(out=indb, in_=ind)
            for tt in range(TT):
                nc.sync.dma_start(out=vt[:, tt, :, DH], in_=onesc)
        wvg = p_wqk.tile([128, ET, 512], BF16, tag="wqk", bufs=2, name="wvg")
        nc.sync.dma_start(
            out=wvg, in_=wvT[:, jh * 512:(jh + 1) * 512].rearrange(
                "(e p) j -> p e j", p=128))
        bvv = bvb.rearrange("p (h d) -> p h d", d=DH)
        for tt in range(TT):
            ps = psum_mm.tile([128, 512], F32, tag="mm", bufs=2, name="pmm")
            for et in range(ET):
                nc.tensor.matmul(
                    ps,
                    xt[:, et, tt * 128:(tt + 1) * 128],
                    wvg[:, et, :],
                    start=(et == 0), stop=(et == ET - 1))
                yield
            nc.vector.tensor_add(
                out=vt[:, tt, jh * 8:(jh + 1) * 8, 0:DH],
                in0=ps.rearrange("p (h d) -> p h d", d=DH),
                in1=bvv[:, jh * 8:(jh + 1) * 8, :])

    def w34_loader():
        """Prefetch out_proj + c_proj weights during S2 (long before use),
        one 2MB DMA per matrix."""
        if PHASE_LIMIT < 4:
            return
        for dst, src in ((wout_tiles, woutT), (wc_tiles, wcT)):
            for og in range(2):
                wt = p_w34.tile([128, ET, 512], BF16, tag="w34", bufs=4,
                                name="wt3")
                nc.sync.dma_start(
                    out=wt, in_=src[:, og * 512:(og + 1) * 512].rearrange(
                        "(e p) j -> p e j", p=128))
                dst.append(wt)
                yield

    # ---- attention generator (yields once per tk-iteration) ---------------
    def att_gen(c, a, p_esc):
        cs = c * W
        last_it = 2 * c + 1
        avps = [psum.tile([128, W], F32, tag=f"av{p}", bufs=1,
                          name=f"avp{p}") for p in range(2)]
        pend = []

        def emit_av(it, sub, clen, esc):
            for p in range(2):
                nc.tensor.matmul(
                    avps[p][0:DH + 1, sub:sub + clen],
                    vt[:, it, 2 * a + p, :],
                    esc[:, p, :clen],
                    start=(it == 0), stop=(it == last_it),
                    skip_group_check=True)

        for it in range(last_it + 1):
            n0 = it * 128
            lo = max(n0, cs)
            sub = lo - cs
            clen = W - sub
            scp = psum.tile([128, 2, W], F32, tag="scp", bufs=2, name="scp",
                            padded_shape=[128, 2, 512])
            for p in range(2):             # paired heads: row-tiled matmuls
                pb = p * 64
                nc.tensor.matmul(
                    scp[:, p, :clen],
                    qkt[pb:pb + 64, KJ0 + a, n0:n0 + 128],
                    qkt[pb:pb + 64, a, lo:lo + clen],
                    start=True, stop=True)
            esc = p_esc.tile([128, 2, W], BF16, tag="esc", name="esc")
            nc.scalar.activation(out=esc[:, :, :clen], in_=scp[:, :, :clen],
                                 func=Act.Exp, scale=1.0 / 8.0)
            if n0 >= cs:                   # diagonal block: causal mask
                nc.vector.tensor_mul(
                    esc[:, :, 0:128], esc[:, :, 0:128],
                    mask01b[:, None, :].broadcast_to([128, 2, 128]))
            pend.append((it, sub, clen, esc))
            if len(pend) > LAG:
                emit_av(*pend.pop(0))
            yield
        for args in pend:
            emit_av(*args)
        # stage the denominator rows at partition 64 (engines address
        # partition bases in multiples of 32) and DMA-scatter (partition-
        # agnostic) into this chunk's [16, W] denominator tile.
        stg = p_nrm.tile([128, 2, W], F32, tag="stg", bufs=2, name="stg")
        for p in range(2):                 # drain unnormalized y + denom row
            nc.vector.tensor_copy(out=yt[p * 64:p * 64 + 64, a, cs:cs + W],
                                  in_=avps[p][0:DH, :])
            nc.vector.tensor_copy(out=stg[64:65, p, :],
                                  in_=avps[p][DH:DH + 1, :])
        hb, ar = divmod(a, 4)
        nc.sync.dma_start(out=denc[hb][2 * ar:2 * ar + 2, c, :],
                          in_=stg[64:65, :, :])

    def emit_rec(c, hb):
        """1/denom for chunk c, pair-half hb (batched DVE reciprocal).
        Emitted directly (not via the dense FIFO): the lo half right at S3
        chunk start (its denominators date from S2), the hi half the moment
        the chunk's last unit drains, hiding its latency under the lo-half
        normalize matmuls."""
        with nc.allow_low_precision(reason="fp32 reciprocal feeding a bf16 "
                                    "multiply; well inside tolerance"):
            nc.vector.reciprocal(out=recc[hb][:, c, :], in_=denc[hb][:, c, :])

    def norm_gen(c):
        """Broadcast 1/denom across partitions by K=16 indicator matmuls,
        apply in place on DVE."""
        cs = c * W
        for a in range(H // 2):
            hb, ar = divmod(a, 4)
            rb = psum_mo.tile([128, W], F32, tag="mo", bufs=2, name="rb")
            nc.tensor.matmul(rb, indb[:, ar, :], recc[hb][:, c, :],
                             start=True, stop=True)
            yield
            nc.vector.tensor_mul(yt[:, a, cs:cs + W], yt[:, a, cs:cs + W],
                                 rb)
            yield

    def oproj_gen(c):
        cs = c * W
        for ot in range(ET):
            og, os_ = divmod(ot, 4)
            ps = psum_mo.tile([128, W], F32, tag="mo", bufs=2, name="po")
            for et in range(ET):
                nc.tensor.matmul(
                    ps,
                    wout_tiles[og][:, et, os_ * 128:(os_ + 1) * 128],
                    yt[:, et, cs:cs + W],
                    start=(et == 0), stop=(et == ET - 1))
                yield
            if ot % 2 == 0:
                nc.scalar.activation(out=zt[:, ot, cs:cs + W], in_=ps,
                                     func=Act.Identity,
                                     bias=boutb[:, ot:ot + 1], scale=1.0)
            else:
                nc.vector.tensor_scalar_add(out=zt[:, ot, cs:cs + W],
                                            in0=ps,
                                            scalar1=boutb[:, ot:ot + 1])

    def cproj_gen(c):
        cs = c * W
        for ot in range(ET):
            og, os_ = divmod(ot, 4)
            ps = psum_mo.tile([128, W], F32, tag="mo", bufs=2, name="pc")
            for et in range(ET):
                nc.tensor.matmul(
                    ps,
                    wc_tiles[og][:, et, os_ * 128:(os_ + 1) * 128],
                    zt[:, et, cs:cs + W],
                    start=(et == 0), stop=(et == ET - 1))
                yield
            ob = p_out.tile([128, W], F32, tag="ob", bufs=3, name="ob")
            if ot % 2 == 0:
                nc.vector.tensor_scalar_add(out=ob, in0=ps,
                                            scalar1=bcb[:, ot:ot + 1])
            else:
                nc.scalar.activation(out=ob, in_=ps, func=Act.Identity,
                                     bias=bcb[:, ot:ot + 1], scale=1.0)
            nc.sync.dma_start(out=outT[ot * 128:(ot + 1) * 128, cs:cs + W],
                              in_=ob)

    # ---- drivers ----------------------------------------------------------
    def run_dense(dense, n=None):
        steps = 0
        while dense and (n is None or steps < n):
            try:
                next(dense[0])
                steps += 1
            except StopIteration:
                dense.pop(0)
        return steps

    def drive(att_units, dense, ratio):
        att_units = list(att_units)
        while att_units:
            try:
                next(att_units[0])
            except StopIteration:
                att_units.pop(0)
                continue
            run_dense(dense, ratio)
        run_dense(dense)

    wout_tiles = []
    wc_tiles = []

    # S1: dense-only warmup — deps for attention pairs 0-3
    dense1 = [qk_gen(0), qk_gen(2)] + ([vb_gen(0)] if PHASE_LIMIT >= 2 else [])
    run_dense(dense1)

    # S2: attention pairs 0-3 (all chunks) over the remaining qkv work,
    # with out/c_proj weight prefetch at the back of the DMA queue
    p_esc1 = tc.alloc_tile_pool(name="p_esc1", bufs=6)
    dense2 = [qk_gen(1), qk_gen(3)] + ([vb_gen(1)] if PHASE_LIMIT >= 2 else [])
    dense2.append(w34_loader())
    att2 = [att_gen(c, a, p_esc1)
            for a in range(4) for c in range(NT)] if PHASE_LIMIT >= 3 else []
    drive(att2, dense2, RATIO_S2)
    p_esc1.release()
    psum_mm.release()
    p_wqk.release()
    p_x.release()

    # S3/S4: attention pairs 4-7 chunk by chunk; after each chunk completes,
    # its normalize + out_proj + c_proj join the dense stream
    psum_mo = tc.alloc_tile_pool(name="psum_mo", bufs=1, space="PSUM")
    p_esc2 = tc.alloc_tile_pool(name="p_esc2", bufs=6)
    p_out = tc.alloc_tile_pool(name="p_out", bufs=3)
    dense3 = []
    if PHASE_LIMIT >= 3:
        for c in range(NT):
            if PHASE_LIMIT >= 4:
                emit_rec(c, 0)
            drive([att_gen(c, a, p_esc2) for a in range(4, 8)],
                  dense3, RATIO_S3)
            if PHASE_LIMIT >= 4:
                emit_rec(c, 1)
                dense3.append(norm_gen(c))
                dense3.append(oproj_gen(c))
                dense3.append(cproj_gen(c))
    run_dense(dense3)

    p_out.release()
    p_esc2.release()
    psum_mo.release()
    p_w34.release()
    p_z.release()
    p_v.release()
    p_qk.release()
    p_y.release()
    p_nrm.release()
    p_den.release()
    psum.release()
    consts.release()


def _build():
    if "nc" in _CACHE:
        return _CACHE["nc"]
    nc = bacc.Bacc("TRN2", target_bir_lowering=False, debug=False,
                   enable_asserts=True, num_devices=8)
    d = nc.dram_tensor
    aps = [
        d("xT", [E, T], BF16, kind="ExternalInput").ap(),
        d("wqkT", [E, JQK], BF16, kind="ExternalInput").ap(),
        d("wvT", [E, E], BF16, kind="ExternalInput").ap(),
        d("bqk", [128, JQK // 128], F32, kind="ExternalInput").ap(),
        d("bvrow", [1, E], BF16, kind="ExternalInput").ap(),
        d("woutT", [E, E], BF16, kind="ExternalInput").ap(),
        d("bout", [128, E // 128], F32, kind="ExternalInput").ap(),
        d("wcT", [E, E], BF16, kind="ExternalInput").ap(),
        d("bc", [128, E // 128], F32, kind="ExternalInput").ap(),
        d("mask01", [128, 128], BF16, kind="ExternalInput").ap(),
        d("onesbf", [128, H], BF16, kind="ExternalInput").ap(),
        d("ind", [8, (H // 4) * 128], BF16, kind="ExternalInput").ap(),
        d("outT", [E, T], F32, kind="ExternalOutput").ap(),
    ]
    with tile.TileContext(nc) as tc:
        _emit(nc, tc, aps)
    nc.compile()
    _CACHE["nc"] = nc
    return nc


def _host_inputs(x, in_proj_w, in_proj_b, out_proj_w, out_proj_b,
                 c_proj_w, c_proj_b):
    f = np.float32
    bf = ml_dtypes.bfloat16
    x = np.asarray(x, f)
    in_proj_w = np.asarray(in_proj_w, f)
    in_proj_b = np.asarray(in_proj_b, f)
    ind = np.zeros((8, H // 4, 128), f)
    for a in range(H // 4):
        ind[2 * a, a, 0:64] = 1.0
        ind[2 * a + 1, a, 64:128] = 1.0
    shared = {
        "wqkT": np.ascontiguousarray(in_proj_w[:JQK].T).astype(bf),
        "wvT": np.ascontiguousarray(in_proj_w[JQK:].T).astype(bf),
        "bqk": np.ascontiguousarray(
            in_proj_b[:JQK].reshape(JQK // 128, 128).T),
        "bvrow": in_proj_b[JQK:].reshape(1, E).astype(bf),
        "woutT": np.ascontiguousarray(np.asarray(out_proj_w, f).T).astype(bf),
        "bout": np.ascontiguousarray(
            np.asarray(out_proj_b, f).reshape(E // 128, 128).T),
        "wcT": np.ascontiguousarray(np.asarray(c_proj_w, f).T).astype(bf),
        "bc": np.ascontiguousarray(
            np.asarray(c_proj_b, f).reshape(E // 128, 128).T),
        "mask01": np.where(np.arange(128)[None, :] >= np.arange(128)[:, None],
                           f(1.0), f(0.0)).astype(bf),
        "onesbf": np.ones((128, H), bf),
        "ind": np.ascontiguousarray(ind.reshape(8, (H // 4) * 128)).astype(
            bf),
    }
    return [{**shared, "xT": np.ascontiguousarray(x[b].T).astype(bf)}
            for b in range(B)]


def kernel(x, in_proj_w, in_proj_b, out_proj_w, out_proj_b, c_proj_w,
           c_proj_b):
    nc = _build()
    in_maps = _host_inputs(x, in_proj_w, in_proj_b, out_proj_w, out_proj_b,
                           c_proj_w, c_proj_b)
    res = run_bass_kernel_spmd(nc, in_maps, core_ids=list(range(B)),
                               trace=TRACE)
    _CACHE["last_result"] = res
    out = np.stack([res.results[b]["outT"].T for b in range(B)])
    return np.ascontiguousarray(out, dtype=np.float32)
